# revision 1
# baseline (speedup 1.0000x reference)
"""Trainium2 Bass kernel for BaseLayerWithLoRA.

Computes out = x @ W.T + bias + (x @ A.T) @ B.T for
x [2, 4096, 4096], W [4096, 4096], bias [4096], A [16, 4096], B [4096, 16].

Strategy
--------
The LoRA path is rank-16, so it folds into the base weight on the host:

    W' = W + B @ A        (fp32, host)
    out = x @ W'.T + bias (device: one GEMM + per-partition bias add)

Device math runs in fp16 (fp32 PSUM accumulation): rel-err ~4e-4,
well inside the 2e-2 gate, and it halves HBM traffic vs fp32.

Sharding: data-parallel over tokens (8192 -> 1024/core). Each core keeps
its x.T shard (8 MiB fp16) resident in SBUF and streams W' (32 MiB fp16)
exactly once.

Layout: W' tiles are the stationary operand ([128 d_in, 128 d_out]), x.T
tiles the moving operand ([128 d_in, 512 tokens]) -> PSUM [128 d_out,
512 tok]; the output is produced transposed (outT [d_out, tok]) and
un-transposed on the host. Panels are processed in groups of 4,
t-interleaved, so all 8 PSUM banks accumulate concurrently and the PE
never waits on the x-shard DMA during ramp-up. Bias is added during
PSUM eviction (scalar engine for even banks, vector for odd) and the
result is stored via the sync HWDGE queue.
"""

import os
import sys

for _p in ("/opt/trn_rl_repo", "/opt/pypackages"):
    if _p not in sys.path:
        sys.path.append(_p)

# The kernel executes on the axon-tunneled NeuronCores via PJRT; a
# JAX_PLATFORMS=cpu pin (used by some reference harnesses) would hide them.
_jp = os.environ.get("JAX_PLATFORMS")
if _jp and "axon" not in _jp:
    del os.environ["JAX_PLATFORMS"]

import numpy as np
import concourse.bacc as bacc
import concourse.mybir as mybir
from concourse.tile import TileContext
from concourse.bass_utils import run_bass_kernel_spmd

F32 = mybir.dt.float32
# fp16: same PE rate as bf16, 10-bit mantissa, and no periodic beat skips
F16 = mybir.dt.float16
NP_F16 = mybir.dt.np(F16)

BATCH, SEQ, D_IN, D_OUT, RANK = 2, 4096, 4096, 4096, 16
N_CORES = 8
TOK = BATCH * SEQ            # 8192 tokens total
TOK_C = TOK // N_CORES       # 1024 tokens per core
P = 128                      # partitions
NT = D_IN // P               # 32 contraction (k) tiles
NPO = D_OUT // P             # 32 output panels of 128 features
H = TOK_C // 512             # 2 moving-operand blocks of 512 tokens
GRP = 4                      # panels interleaved t-major per group
NG = NPO // GRP              # 8 groups

_NC_CACHE = None


def _build_nc():
    """Trace + schedule + compile the per-core Bass module (SPMD: all 8
    cores run this same program on their own token shard)."""
    nc = bacc.Bacc(None, target_bir_lowering=False, debug=False)

    xT = nc.dram_tensor("xT", [D_IN, TOK_C], F16, kind="ExternalInput")
    # W' pre-packed on host: Wp[p, (po*NT + t)*P + m] = W'[po*P+m, t*P+p]
    # so each 128-feature panel is one contiguous [128, NT*P] slab.
    Wp = nc.dram_tensor("Wp", [P, NPO * NT * P], F16, kind="ExternalInput")
    biasT = nc.dram_tensor("biasT", [P, NPO], F32, kind="ExternalInput")
    outT = nc.dram_tensor("outT", [D_OUT, TOK_C], F32, kind="ExternalOutput")

    xT_t = xT.rearrange("(t p) n -> t p n", p=P)

    with TileContext(nc) as tc:
        with (
            tc.tile_pool(name="xpool", bufs=1) as xpool,
            tc.tile_pool(name="cpool", bufs=1) as cpool,
            tc.tile_pool(name="wpool", bufs=16) as wpool,
            tc.tile_pool(name="wpool0", bufs=1) as wpool0,
            tc.tile_pool(name="opool", bufs=8) as opool,
            tc.tile_pool(name="pspool", bufs=1, space="PSUM") as pspool,
        ):
            # W' panels stream on the scalar ring in quarter-panel chunks
            # (the first matmul only waits on 256 KiB, not a full panel).
            # 16 rotating buffers = one group of prefetch: buffer-reuse
            # deps throttle W to 4 MiB during the x-shard load, so the x
            # stream keeps ~2/3 of HBM bandwidth during ramp-up.
            CH = NT // 4  # k-tiles per chunk
            # The last 4 panels run as two 2-panel groups (see below); the
            # chunk stream and the matmul loop share this schedule.
            schedule = [(g * GRP, GRP, 0) for g in range(NG - 1)]
            schedule += [(NPO - 4, 2, 0), (NPO - 2, 1, 4), (NPO - 1, 1, 6)]
            # Emit chunks in exact consumption order (group, quarter,
            # panel) so the ramp never waits on an out-of-order chunk.
            # Group 0's first quarter goes out as 2-k-tile minichunks,
            # panel-interleaved: the first real matmul is then gated on
            # 4x64 KiB instead of 4x256 KiB and starts right as the
            # HAM warm-up dummies finish.
            wmini = {}
            for m in range(4):
                for j in range(GRP):
                    wm = wpool0.tile([P, 2 * P], F16, name=f"wm{j}_{m}", tag=f"wm{j}_{m}")
                    base = j * NT * P + m * 2 * P
                    nc.scalar.dma_start(out=wm[:], in_=Wp[:, base : base + 2 * P])
                    wmini[j, m] = wm
            wch = {}
            for p0, npan, _ in schedule:
                for c in range(4):
                    if p0 == 0 and c == 0:
                        continue
                    for j in range(npan):
                        po = p0 + j
                        wt = wpool.tile(
                            [P, CH * P], F16, name=f"wt{po}_{c}", tag="wt"
                        )
                        base = po * NT * P + c * CH * P
                        nc.scalar.dma_start(
                            out=wt[:], in_=Wp[:, base : base + CH * P]
                        )
                        wch[po, c] = wt

            # Resident x.T shard on the sync ring: 32 tiles [128, 1024].
            xts = []
            for t in range(NT):
                xt = xpool.tile([P, TOK_C], F16, name=f"xt{t}", tag=f"xt{t}")
                nc.sync.dma_start(out=xt[:], in_=xT_t[t])
                xts.append(xt)

            # Bias rides the sync ring behind x (lands ~25us, first use
            # ~65us); keeping it off the scalar ring lets the first W'
            # chunk start sooner.
            bias_sb = cpool.tile([P, NPO], F32, name="bias_sb", tag="bias_sb")
            nc.sync.dma_start(out=bias_sb[:], in_=biasT[:])

            # HAM pre-warm: the PE sits data-starved for ~5us while the
            # first W' chunks + x tiles land, then pays ~2.5us of 1.2GHz
            # cold-clock matmuls. Burn that cold window early with dummy
            # matmuls on a memset scratch tile; they finish before the
            # first real operands arrive, so the real stream starts warm.
            scratch = cpool.tile([P, 512], F16, name="scratch", tag="scratch")
            nc.vector.memset(scratch[:], 0)
            ps_warm = pspool.tile([P, 512], F32, name="ps_warm", tag="ps0")
            for i in range(10):
                nc.tensor.matmul(
                    ps_warm[:],
                    scratch[:, 0:P],
                    scratch[:],
                    start=True,
                    stop=True,
                    skip_group_check=True,
                )

            # 8 PSUM banks: groups of 4 panels x 2 token blocks accumulate
            # concurrently (t-major), so the PE keeps up with the x DMA
            # from the first tile onward. The last 4 panels run as two
            # 2-panel groups on disjoint bank sets so the final
            # evictions + stores drain in half the time.
            for gi, (p0, npan, boff) in enumerate(schedule):
                psums = {}
                for j in range(npan):
                    for h in range(H):
                        b = boff + j * H + h
                        psums[b] = pspool.tile(
                            [P, 512], F32, name=f"ps_{gi}_{b}", tag=f"ps{b}"
                        )
                for t in range(NT):
                    for j in range(npan):
                        po = p0 + j
                        if p0 == 0 and t < CH:
                            w = wmini[j, t // 2][:, (t % 2) * P : (t % 2 + 1) * P]
                        else:
                            w = wch[po, t // CH][:, (t % CH) * P : (t % CH + 1) * P]
                        for h in range(H):
                            nc.tensor.matmul(
                                psums[boff + j * H + h][:],
                                w,
                                xts[t][:, h * 512 : (h + 1) * 512],
                                start=(t == 0),
                                stop=(t == NT - 1),
                            )
                for j in range(npan):
                    po = p0 + j
                    for h in range(H):
                        b = boff + j * H + h
                        ot = opool.tile([P, 512], F32, name=f"ot_{gi}_{b}", tag="ot")
                        # Bias-add during eviction; split banks across the
                        # scalar and vector engines so they drain in parallel.
                        if h == 0:
                            nc.scalar.add(ot[:], psums[b][:], bias_sb[:, po : po + 1])
                        else:
                            nc.vector.tensor_scalar_add(
                                ot[:], psums[b][:], bias_sb[:, po : po + 1]
                            )
                        # Alternate store rings so the final group's stores
                        # drain on two HWDGE rings (halves the tail).
                        ring = nc.sync if h == 0 else nc.scalar
                        ring.dma_start(
                            out=outT[po * P : (po + 1) * P, h * 512 : (h + 1) * 512],
                            in_=ot[:],
                        )

    nc.compile()
    return nc


def _get_nc():
    global _NC_CACHE
    if _NC_CACHE is None:
        _NC_CACHE = _build_nc()
    return _NC_CACHE


def _prep_inputs(x, W, bias, A, B):
    """Host-side fold + layout prep + sharding. Returns per-core inputs."""
    x_flat = np.asarray(x, dtype=np.float32).reshape(TOK, D_IN)
    Wf = np.asarray(W, dtype=np.float32) + np.asarray(B, dtype=np.float32) @ np.asarray(
        A, dtype=np.float32
    )
    # Pack W'.T into per-panel stationary-tile slabs (see _build_nc).
    # Wp[p, ((po*NT)+t)*P+m] = W'T[t*P+p, po*P+m]
    Wp = np.ascontiguousarray(
        np.ascontiguousarray(Wf.T)
        .reshape(NT, P, NPO, P)
        .transpose(1, 2, 0, 3)
        .reshape(P, NPO * NT * P)
        .astype(NP_F16)
    )
    biasT = np.ascontiguousarray(
        np.asarray(bias, dtype=np.float32).reshape(NPO, P).T
    )
    x_bf = x_flat.astype(NP_F16)
    in_maps = []
    for c in range(N_CORES):
        xT_c = np.ascontiguousarray(x_bf[c * TOK_C : (c + 1) * TOK_C, :].T)
        in_maps.append({"xT": xT_c, "Wp": Wp, "biasT": biasT})
    return in_maps


def _run(inputs, trace=False, trace_cores=None):
    nc = _get_nc()
    in_maps = _prep_inputs(**inputs)
    res = run_bass_kernel_spmd(
        nc,
        in_maps,
        core_ids=list(range(N_CORES)),
        trace=trace,
        trace_cores=trace_cores,
    )
    full = np.empty((TOK, D_OUT), dtype=np.float32)
    for c in range(N_CORES):
        full[c * TOK_C : (c + 1) * TOK_C, :] = res.results[c]["outT"].T
    return full.reshape(BATCH, SEQ, D_OUT), res


def kernel(**inputs):
    full, _ = _run(inputs, trace=False)
    return full


if __name__ == "__main__":
    rng = np.random.default_rng(0)
    inputs = {
        "x": rng.standard_normal((BATCH, SEQ, D_IN), dtype=np.float32),
        "W": rng.standard_normal((D_OUT, D_IN), dtype=np.float32) * 0.02,
        "bias": rng.standard_normal((D_OUT,), dtype=np.float32) * 0.02,
        "A": rng.standard_normal((RANK, D_IN), dtype=np.float32) * 0.02,
        "B": rng.standard_normal((D_OUT, RANK), dtype=np.float32) * 0.02,
    }
    got = kernel(**inputs)
    x64 = inputs["x"].reshape(TOK, D_IN).astype(np.float64)
    exp = x64 @ inputs["W"].astype(np.float64).T + inputs["bias"]
    exp += (x64 @ inputs["A"].astype(np.float64).T) @ inputs["B"].astype(np.float64).T
    exp = exp.reshape(BATCH, SEQ, D_OUT)
    rel = np.linalg.norm(got - exp) / np.linalg.norm(exp)
    print("self-check relative error:", rel)



# revision 2
# speedup vs baseline: 1.0953x; 1.0953x over previous
"""Trainium2 Bass kernel for BaseLayerWithLoRA.

Computes out = x @ W.T + bias + (x @ A.T) @ B.T for
x [2, 4096, 4096], W [4096, 4096], bias [4096], A [16, 4096], B [4096, 16].

Strategy
--------
The LoRA path is rank-16, so it folds into the base weight on the host:

    W' = W + B @ A        (fp32, host)
    out = x @ W'.T + bias (device: one GEMM + fused scale/bias eviction)

Mixed-precision k-split: the PE runs fp16 at 1 MAC/cycle and fp8
(e4m3, DoubleRow perf mode) at 2 MACs/cycle. Putting a fraction f of
the K=4096 contraction through fp8 cuts PE time by f/2 while the
rel-err grows as ~3.75%*sqrt(f) (both operands e4m3). With f = 3/16
(k-tiles 26..31 of 32) the measured error lands ~1.6e-2 vs the 2e-2
gate, and PE time drops ~9%.

Both paths accumulate into one PSUM group at a common x512 scale:
W'x512 is exact in fp16 (exponent shift) and centers the e4m3
quantization grid (std ~10, no subnormals). Eviction fuses
out = psum*(1/512) + bias in a single scalar/vector instruction.

Sharding: data-parallel over tokens (8192 -> 1024/core). Each core keeps
its x shard resident in SBUF (fp16 k<3328, e4m3 pair-tiles k>=3328) and
streams W' exactly once (fp16 26/32 + e4m3 6/32 = ~29 MiB).

Layout: W' tiles are the stationary operand, x the moving operand
([128 k, 512 tokens] -> PSUM [128 d_out, 512 tok]); output is produced
transposed and un-transposed on the host. Panels run in groups of 4,
t-interleaved over all 8 PSUM banks. fp8 DoubleRow tiles ([128, 2, 128]
stationary x [128, 2, 512] moving) close each accumulation chain.
"""

import os
import sys

for _p in ("/opt/trn_rl_repo", "/opt/pypackages"):
    if _p not in sys.path:
        sys.path.append(_p)

# The kernel executes on the axon-tunneled NeuronCores via PJRT; a
# JAX_PLATFORMS=cpu pin (used by some reference harnesses) would hide them.
_jp = os.environ.get("JAX_PLATFORMS")
if _jp and "axon" not in _jp:
    del os.environ["JAX_PLATFORMS"]

import numpy as np
import concourse.bacc as bacc
import concourse.mybir as mybir
from concourse.tile import TileContext
from concourse.bass_utils import run_bass_kernel_spmd

F32 = mybir.dt.float32
F16 = mybir.dt.float16
F8 = mybir.dt.float8e4          # e4m3: 2x PE rate in DoubleRow perf mode
NP_F16 = mybir.dt.np(F16)
NP_F8 = mybir.dt.np(F8)
DR = mybir.MatmulPerfMode.DoubleRow
IDENT = mybir.ActivationFunctionType.Identity
MULT = mybir.AluOpType.mult
ADD = mybir.AluOpType.add

BATCH, SEQ, D_IN, D_OUT, RANK = 2, 4096, 4096, 4096, 16
N_CORES = 8
TOK = BATCH * SEQ            # 8192 tokens total
TOK_C = TOK // N_CORES       # 1024 tokens per core
P = 128                      # partitions
NT = D_IN // P               # 32 contraction (k) tiles total
NT16 = 26                    # k-tiles 0..25 in fp16
NP8 = (NT - NT16) // 2       # 3 fp8 DoubleRow pair-tiles (k-tiles 26..31)
K16 = NT16 * P               # 3328 fp16 contraction rows
NPO = D_OUT // P             # 32 output panels of 128 features
H = TOK_C // 512             # 2 moving-operand blocks of 512 tokens
GRP = 4                      # panels interleaved t-major per group
NG = NPO // GRP              # 8 groups
SCALE = 512.0                # W' pre-scale (exact in fp16; centers e4m3)
INV = 1.0 / SCALE

_NC_CACHE = None


def _build_nc():
    """Trace + schedule + compile the per-core Bass module (SPMD: all 8
    cores run this same program on their own token shard)."""
    nc = bacc.Bacc(None, target_bir_lowering=False, debug=False)

    xT16 = nc.dram_tensor("xT16", [K16, TOK_C], F16, kind="ExternalInput")
    # fp8 pair-tiles: x8[kk, p, i, n] = e4m3(x[n, K16 + kk*256 + i*128 + p])
    x8d = nc.dram_tensor("x8", [NP8, P, 2, TOK_C], F8, kind="ExternalInput")
    # W' fp16 slab: Wp16[p, (po*NT16 + t)*P + m] = 512*W'[po*P+m, t*P+p]
    Wp16 = nc.dram_tensor("Wp16", [P, NPO * NT16 * P], F16, kind="ExternalInput")
    # W' fp8 pair-tiles: Wp8[p, po, kk, i, m] = e4m3(512*W'[po*P+m, K16+kk*256+i*128+p])
    Wp8 = nc.dram_tensor("Wp8", [P, NPO, NP8, 2, P], F8, kind="ExternalInput")
    biasT = nc.dram_tensor("biasT", [P, NPO], F32, kind="ExternalInput")
    outT = nc.dram_tensor("outT", [D_OUT, TOK_C], F32, kind="ExternalOutput")

    xT_t = xT16.rearrange("(t p) n -> t p n", p=P)

    with TileContext(nc) as tc:
        with (
            tc.tile_pool(name="xpool", bufs=1) as xpool,
            tc.tile_pool(name="cpool", bufs=1) as cpool,
            tc.tile_pool(name="wpool", bufs=16) as wpool,
            tc.tile_pool(name="wpool0", bufs=1) as wpool0,
            tc.tile_pool(name="opool", bufs=8) as opool,
            tc.tile_pool(name="pspool", bufs=1, space="PSUM") as pspool,
        ):
            # W' panels stream on the scalar ring in chunks; 16 rotating
            # buffers throttle prefetch so the x-shard load keeps most of
            # the HBM bandwidth during ramp-up.
            # The last 4 panels run as two 2-panel groups (see below); the
            # chunk stream and the matmul loop share this schedule.
            schedule = [(g * GRP, GRP, 0) for g in range(NG - 1)]
            schedule += [(NPO - 4, 2, 0), (NPO - 2, 1, 4), (NPO - 1, 1, 6)]
            # Group 0's first 8 k-tiles go out as 2-k-tile minichunks,
            # panel-interleaved: the first real matmul is then gated on
            # 4x64 KiB and starts right as the HAM warm-up dummies finish.
            wmini = {}
            for m in range(4):
                for j in range(GRP):
                    wm = wpool0.tile([P, 2 * P], F16, name=f"wm{j}_{m}", tag=f"wm{j}_{m}")
                    base = j * NT16 * P + m * 2 * P
                    nc.scalar.dma_start(out=wm[:], in_=Wp16[:, base : base + 2 * P])
                    wmini[j, m] = wm
            # fp16 chunks c=0..3 cover k-tiles [0:8, 8:16, 16:24, 24:26];
            # chunk c=4 is the panel's fp8 pair-tile slab.
            CSTART = (0, 8, 16, 24)
            CLEN = (8, 8, 8, 2)
            wch = {}
            wch8 = {}
            for p0, npan, _ in schedule:
                for c in range(5):
                    if p0 == 0 and c == 0:
                        continue
                    for j in range(npan):
                        po = p0 + j
                        if c < 4:
                            wt = wpool.tile(
                                [P, CLEN[c] * P], F16, name=f"wt{po}_{c}", tag="wt"
                            )
                            base = (po * NT16 + CSTART[c]) * P
                            nc.scalar.dma_start(
                                out=wt[:], in_=Wp16[:, base : base + CLEN[c] * P]
                            )
                            wch[po, c] = wt
                        else:
                            wt8 = wpool.tile(
                                [P, NP8, 2, P], F8, name=f"w8_{po}", tag="wt"
                            )
                            nc.scalar.dma_start(out=wt8[:], in_=Wp8[:, po])
                            wch8[po] = wt8

            # Resident x shard on the sync ring: 26 fp16 tiles [128, 1024]
            # then 3 e4m3 pair-tiles [128, 2, 1024] (consumption order).
            xts = []
            for t in range(NT16):
                xt = xpool.tile([P, TOK_C], F16, name=f"xt{t}", tag=f"xt{t}")
                nc.sync.dma_start(out=xt[:], in_=xT_t[t])
                xts.append(xt)
            x8ts = []
            for kk in range(NP8):
                x8t = xpool.tile([P, 2, TOK_C], F8, name=f"x8t{kk}", tag=f"x8t{kk}")
                nc.sync.dma_start(out=x8t[:], in_=x8d[kk])
                x8ts.append(x8t)

            # Bias rides the sync ring behind x; keeping it off the scalar
            # ring lets the first W' chunk start sooner.
            bias_sb = cpool.tile([P, NPO], F32, name="bias_sb", tag="bias_sb")
            nc.sync.dma_start(out=bias_sb[:], in_=biasT[:])

            # HAM pre-warm: burn the cold-clock window with dummy matmuls
            # on a memset scratch tile before the first real operands land.
            scratch = cpool.tile([P, 512], F16, name="scratch", tag="scratch")
            nc.vector.memset(scratch[:], 0)
            ps_warm = pspool.tile([P, 512], F32, name="ps_warm", tag="ps0")
            for i in range(10):
                nc.tensor.matmul(
                    ps_warm[:],
                    scratch[:, 0:P],
                    scratch[:],
                    start=True,
                    stop=True,
                    skip_group_check=True,
                )

            # 8 PSUM banks: groups of 4 panels x 2 token blocks accumulate
            # concurrently (t-major). Chain per bank: 26 fp16 matmuls then
            # 3 fp8 DoubleRow matmuls (K=256 each), all scaled x512.
            for gi, (p0, npan, boff) in enumerate(schedule):
                psums = {}
                for j in range(npan):
                    for h in range(H):
                        b = boff + j * H + h
                        psums[b] = pspool.tile(
                            [P, 512], F32, name=f"ps_{gi}_{b}", tag=f"ps{b}"
                        )
                for t in range(NT16):
                    for j in range(npan):
                        po = p0 + j
                        if p0 == 0 and t < 8:
                            w = wmini[j, t // 2][:, (t % 2) * P : (t % 2 + 1) * P]
                        else:
                            c = t // 8 if t < 24 else 3
                            o = t - CSTART[c]
                            w = wch[po, c][:, o * P : (o + 1) * P]
                        for h in range(H):
                            nc.tensor.matmul(
                                psums[boff + j * H + h][:],
                                w,
                                xts[t][:, h * 512 : (h + 1) * 512],
                                start=(t == 0),
                                stop=False,
                            )
                for kk in range(NP8):
                    for j in range(npan):
                        po = p0 + j
                        w8 = wch8[po][:, kk]
                        for h in range(H):
                            nc.tensor.matmul(
                                psums[boff + j * H + h][:],
                                w8,
                                x8ts[kk][:, :, h * 512 : (h + 1) * 512],
                                start=False,
                                stop=(kk == NP8 - 1),
                                perf_mode=DR,
                            )
                for j in range(npan):
                    po = p0 + j
                    for h in range(H):
                        b = boff + j * H + h
                        ot = opool.tile([P, 512], F32, name=f"ot_{gi}_{b}", tag="ot")
                        # Fused out = psum*(1/512) + bias during eviction;
                        # split banks across scalar and vector engines.
                        if h == 0:
                            nc.scalar.activation(
                                ot[:], psums[b][:], IDENT,
                                bias=bias_sb[:, po : po + 1], scale=INV,
                            )
                        else:
                            nc.vector.tensor_scalar(
                                ot[:], psums[b][:], INV,
                                bias_sb[:, po : po + 1], MULT, ADD,
                            )
                        # Alternate store rings so the final group's stores
                        # drain on two HWDGE rings (halves the tail).
                        ring = nc.sync if h == 0 else nc.scalar
                        ring.dma_start(
                            out=outT[po * P : (po + 1) * P, h * 512 : (h + 1) * 512],
                            in_=ot[:],
                        )

    nc.compile()
    return nc


def _get_nc():
    global _NC_CACHE
    if _NC_CACHE is None:
        _NC_CACHE = _build_nc()
    return _NC_CACHE


def _prep_inputs(x, W, bias, A, B):
    """Host-side fold + quantize + layout prep + sharding."""
    x_flat = np.asarray(x, dtype=np.float32).reshape(TOK, D_IN)
    Wf = np.asarray(W, dtype=np.float32) + np.asarray(B, dtype=np.float32) @ np.asarray(
        A, dtype=np.float32
    )
    WsT = np.ascontiguousarray(Wf.T) * SCALE      # [k, m], x512
    Wp16 = np.ascontiguousarray(
        WsT[:K16]
        .reshape(NT16, P, NPO, P)
        .transpose(1, 2, 0, 3)
        .reshape(P, NPO * NT16 * P)
        .astype(NP_F16)
    )
    Wp8 = np.ascontiguousarray(
        WsT[K16:]
        .astype(NP_F8)
        .reshape(NP8, 2, P, NPO, P)
        .transpose(2, 3, 0, 1, 4)
        .reshape(P, NPO, NP8, 2, P)
    )
    biasT = np.ascontiguousarray(
        np.asarray(bias, dtype=np.float32).reshape(NPO, P).T
    )
    x16 = x_flat[:, :K16].astype(NP_F16)
    x8 = x_flat[:, K16:].astype(NP_F8)
    in_maps = []
    for c in range(N_CORES):
        sl = slice(c * TOK_C, (c + 1) * TOK_C)
        xT_c = np.ascontiguousarray(x16[sl].T)
        x8_c = np.ascontiguousarray(
            x8[sl].reshape(TOK_C, NP8, 2, P).transpose(1, 3, 2, 0)
        )
        in_maps.append(
            {"xT16": xT_c, "x8": x8_c, "Wp16": Wp16, "Wp8": Wp8, "biasT": biasT}
        )
    return in_maps


def _run(inputs, trace=False, trace_cores=None):
    nc = _get_nc()
    in_maps = _prep_inputs(**inputs)
    res = run_bass_kernel_spmd(
        nc,
        in_maps,
        core_ids=list(range(N_CORES)),
        trace=trace,
        trace_cores=trace_cores,
    )
    full = np.empty((TOK, D_OUT), dtype=np.float32)
    for c in range(N_CORES):
        full[c * TOK_C : (c + 1) * TOK_C, :] = res.results[c]["outT"].T
    return full.reshape(BATCH, SEQ, D_OUT), res


def kernel(**inputs):
    full, _ = _run(inputs, trace=False)
    return full


if __name__ == "__main__":
    rng = np.random.default_rng(0)
    inputs = {
        "x": rng.standard_normal((BATCH, SEQ, D_IN), dtype=np.float32),
        "W": rng.standard_normal((D_OUT, D_IN), dtype=np.float32) * 0.02,
        "bias": rng.standard_normal((D_OUT,), dtype=np.float32) * 0.02,
        "A": rng.standard_normal((RANK, D_IN), dtype=np.float32) * 0.02,
        "B": rng.standard_normal((D_OUT, RANK), dtype=np.float32) * 0.02,
    }
    got = kernel(**inputs)
    x64 = inputs["x"].reshape(TOK, D_IN).astype(np.float64)
    exp = x64 @ inputs["W"].astype(np.float64).T + inputs["bias"]
    exp += (x64 @ inputs["A"].astype(np.float64).T) @ inputs["B"].astype(np.float64).T
    exp = exp.reshape(BATCH, SEQ, D_OUT)
    rel = np.linalg.norm(got - exp) / np.linalg.norm(exp)
    print("self-check relative error:", rel)
